# revision 17
# baseline (speedup 1.0000x reference)
"""GAT message-passing kernel for 8 Trainium2 NeuronCores.

Problem (hardcoded shapes): B=4, N=4096, Cin=200, HC=128.
    x   = rm @ W.T + b                      (B, N, HC)
    e   = (x@a_src)[:, :, None] + (x@a_dst)[:, None, :]
    e   = leaky_relu(e * adj, 0.2)
    out = softmax(e, -1) @ x                (B, N, HC)

Sharding: 8 cores = batch (4) x query-row halves (2). Each core owns
adj[b, i0:i0+2048, :] (33.5 MB) and produces out[b, i0:i0+2048, :].

Per-core algorithm (transposed score layout [j partitions, i free]):
  - scores decompose: src_i = rm_i . (W.T a_src) + b.(a_src+a_dst),
    dst_j = rm_j . (W.T a_dst);  e_ij = src_i + dst_j.
  - leaky_relu is positively homogeneous and adj in {0,1}, so
    lrelu(e*adj) = adj * lrelu(e): one fused custom DVE op computes
    m = adjT * lrelu(src_bcast + dst) in a single 1x pass.
  - w = exp(m) on ScalarE (exp(0)=1 for masked entries matches the
    reference's mask-by-multiplication semantics).
  - aggregation: U[i, 0:128] = sum_j w_ij * x[j, :], D[i] = U[i, 128]
    via one bf16 matmul chain against x~ = [x | 1]; out = U / D.
  - adj is cast fp32->bf16 during the DMA load (values 0/1 exact) and
    transposed on the PE (1 cyc/row) into PSUM for the DVE mask op.
"""

import sys

if "/opt/trn_rl_repo" not in sys.path:
    sys.path.insert(0, "/opt/trn_rl_repo")

import numpy as np

B, N, Cin, HC = 4, 4096, 200, 128
ROWS = N // 2  # rows of adj per core
NEG_SLOPE = 0.2
NCORES = 8
C1 = 128              # first Cin chunk
C2 = Cin - C1         # second Cin chunk (72)
XW = HC + 1           # x~ width (129)
NJT = N // 128        # 32 j-tiles
PANEL = 512
NPANEL = ROWS // PANEL  # 4
NOWN = ROWS // 128      # 16 own i-tiles

_CACHE = {}


def _register_custom_op():
    """Fused DVE op: out = in1 * leaky_relu(in0 + s0, slope=s1).

    in0 = src broadcast tile (stream), s0 = dst per-partition scalar,
    in1 = transposed adj tile (stream, PSUM), s1 = slope immediate.
    """
    import concourse.dve_ops as dve_ops
    from concourse.dve_spec import Spec, Src0, Src1, C0, C1 as SC1, maxx, lower
    from concourse.dve_uop import DveOpSpec

    name = "GAT_MASKED_LRELU_ANT"
    for op in dve_ops.OPS:
        if op.name == name:
            return op
    _t = Src0 + C0
    body = maxx(_t, _t * SC1) * Src1
    spec = Spec(
        body=body,
        reference=lambda in0, in1, s0, s1, imm2: np.maximum(
            in0 + s0, (in0 + s0) * s1
        )
        * in1,
    )
    row = dve_ops._CUSTOM_DVE_ROW_BASE + len(dve_ops.OPS)
    shas = {}
    for ver in ("v3", "v4"):
        uops = lower(spec, ver=ver)
        shas[ver] = DveOpSpec(name=name, opcode=row, uops=uops, rd1_en=True).sha(ver)
    op = dve_ops.DveOp(name, spec, subdim=False, uops_sha=shas)
    dve_ops.OPS.append(op)
    dve_ops._SUB_OPCODE_FOR_NAME[name] = row
    dve_ops.CUSTOM_DVE_SPECS[name] = spec
    return op


def _build(reps=1, loop=1, ablate=()):
    ablate = frozenset(ablate)
    key = ("nc", reps, loop, ablate)
    if key in _CACHE:
        return _CACHE[key]

    import concourse.mybir as mybir
    import concourse.tile as tile
    from concourse import bacc
    from concourse.masks import make_identity

    GAT_OP = _register_custom_op()
    f32 = mybir.dt.float32
    bf16 = mybir.dt.bfloat16
    AF = mybir.ActivationFunctionType

    nc = bacc.Bacc("TRN2", target_bir_lowering=False, debug=False, num_devices=NCORES)

    rmT = nc.dram_tensor("rmT", [Cin, N], f32, kind="ExternalInput").ap()
    rmoT = nc.dram_tensor("rmoT", [Cin, ROWS], f32, kind="ExternalInput").ap()
    adjs = nc.dram_tensor("adjs", [ROWS, N], f32, kind="ExternalInput").ap()
    WTd = nc.dram_tensor("WTd", [Cin, HC], f32, kind="ExternalInput").ap()
    wvd = nc.dram_tensor("wvd", [Cin, 2], f32, kind="ExternalInput").ap()
    brow = nc.dram_tensor("brow", [1, XW + 1], f32, kind="ExternalInput").ap()
    cscal = nc.dram_tensor("cscal", [1, 1], f32, kind="ExternalInput").ap()
    outd = nc.dram_tensor("out", [ROWS, HC], f32, kind="ExternalOutput").ap()

    import contextlib

    with tile.TileContext(nc) as tc:
      for _rep in range(reps):
       with (tc.For_i(0, loop, 1) if loop > 1 else contextlib.nullcontext()):
        with (
            tc.tile_pool(name="const", bufs=1) as cp,
            tc.tile_pool(name="persist", bufs=1) as pp,
            tc.tile_pool(name="adj", bufs=10) as adjp,
        ):
            # ---------- constants ----------
            ident_f = cp.tile([128, 128], f32)
            make_identity(nc, ident_f[:])
            ident_b = cp.tile([128, 128], bf16)
            nc.vector.tensor_copy(ident_b[:], ident_f[:])
            ones_row = cp.tile([1, 128], f32)
            nc.gpsimd.memset(ones_row[:], 1.0)
            brow_sb = cp.tile([1, XW + 1], f32)
            nc.sync.dma_start(out=brow_sb[:], in_=brow)
            # W^T chunks, score vectors w_src|w_dst, and C precomputed on host
            WT1_sb = cp.tile([128, HC], f32)
            nc.sync.dma_start(out=WT1_sb[:], in_=WTd[0:C1, :])
            WT2_sb = cp.tile([C2, HC], f32)
            nc.sync.dma_start(out=WT2_sb[:], in_=WTd[C1:Cin, :])
            wv1_sb = cp.tile([128, 2], f32)
            nc.sync.dma_start(out=wv1_sb[:], in_=wvd[0:C1, :])
            wv2_sb = cp.tile([C2, 2], f32)
            nc.sync.dma_start(out=wv2_sb[:], in_=wvd[C1:Cin, :])
            c_sb = cp.tile([1, 1], f32)
            nc.sync.dma_start(out=c_sb[:], in_=cscal)

            # ---------- x~, dst, src ----------
            xt_all = pp.tile([128, NJT * XW], bf16)   # x~ = [x | 1] per j-tile
            dst_all = pp.tile([128, NJT], f32)        # dst column per j-tile
            src_row = pp.tile([1, ROWS], f32)
            src_bc = pp.tile([128, ROWS], f32)        # src broadcast along partitions

            with tc.tile_pool(name="xin", bufs=1) as xp, \
                 tc.tile_pool(name="x_ps", bufs=3, space="PSUM") as xps, \
                 tc.tile_pool(name="x_ps2", bufs=3, space="PSUM") as xps2:
                rmT1 = xp.tile([128, N], f32)
                nc.sync.dma_start(out=rmT1[:], in_=rmT[0:C1, :])
                rmT2 = xp.tile([C2, N], f32)
                nc.sync.dma_start(out=rmT2[:], in_=rmT[C1:Cin, :])
                rmoT1 = xp.tile([128, ROWS], f32)
                nc.sync.dma_start(out=rmoT1[:], in_=rmoT[0:C1, :])
                rmoT2 = xp.tile([C2, ROWS], f32)
                nc.sync.dma_start(out=rmoT2[:], in_=rmoT[C1:Cin, :])

                for n in range(NJT):
                    sl = slice(n * 128, (n + 1) * 128)
                    # cols 0:129 = x~ = [x | 1]; col 129 = dst.  One
                    # accumulation group seeded by the K=1 bias matmul.
                    x_ps = xps2.tile([128, XW + 1], f32, tag="xps")
                    nc.tensor.matmul(x_ps[:], ones_row[:], brow_sb[:], start=True, stop=False)
                    nc.tensor.matmul(x_ps[:, 0:HC], rmT1[:, sl], WT1_sb[:], start=False, stop=False)
                    nc.tensor.matmul(x_ps[:, 0:HC], rmT2[:, sl], WT2_sb[:], start=False, stop=False)
                    nc.tensor.matmul(x_ps[:, XW:XW + 1], rmT1[:, sl], wv1_sb[:, 1:2], start=False, stop=False)
                    nc.tensor.matmul(x_ps[:, XW:XW + 1], rmT2[:, sl], wv2_sb[:, 1:2], start=False, stop=True)
                    nc.vector.tensor_copy(xt_all[:, n * XW:(n + 1) * XW], x_ps[:, 0:XW])
                    nc.vector.tensor_copy(dst_all[:, n:n + 1], x_ps[:, XW:XW + 1])

                for k in range(NOWN):
                    sl = slice(k * 128, (k + 1) * 128)
                    s_ps = xps2.tile([1, 128], f32, tag="xps", name="s_ps")
                    nc.tensor.matmul(s_ps[:], wv1_sb[:, 0:1], rmoT1[:, sl], start=True, stop=False)
                    nc.tensor.matmul(s_ps[:], wv2_sb[:, 0:1], rmoT2[:, sl], start=False, stop=False)
                    nc.tensor.matmul(s_ps[:], c_sb[:], ones_row[:], start=False, stop=True)
                    nc.scalar.copy(src_row[:, k * 128:(k + 1) * 128], s_ps[:])

                for q in range(ROWS // 512):
                    sb_ps = xps.tile([128, 512], f32, tag="rT", name="sb_ps")
                    nc.tensor.matmul(sb_ps[:], ones_row[:], src_row[:, q * 512:(q + 1) * 512], start=True, stop=True)
                    nc.vector.tensor_copy(src_bc[:, q * 512:(q + 1) * 512], sb_ps[:])

            # ---------- main loop ----------
            with (
                tc.tile_pool(name="mbuf", bufs=4) as mwp,
                tc.tile_pool(name="wbuf", bufs=4) as wxp,
                tc.tile_pool(name="fin", bufs=4) as finp,
                tc.tile_pool(name="U_ps", bufs=4, space="PSUM") as upsp,
                tc.tile_pool(name="aT_ps", bufs=4, space="PSUM") as atp,
            ):
                for p in range(NPANEL):
                    strips = []
                    for s in range(4):
                        r0 = p * PANEL + s * 128
                        if "adjf32" in ablate:
                            at = adjp.tile([128, N // 2], f32, tag="adj", name=f"af_{p}_{s}")
                            nc.sync.dma_start(out=at[:].bitcast(bf16), in_=adjs[r0:r0 + 128, 0:N // 2].bitcast(bf16))
                        elif "adjf32g" in ablate:
                            at = adjp.tile([128, N // 2], f32, tag="adj", name=f"ag_{p}_{s}")
                            nc.gpsimd.dma_start(out=at[:], in_=adjs[r0:r0 + 128, 0:N // 2])
                        else:
                            at = adjp.tile([128, N], bf16, tag="adj")
                            if "noadj" in ablate:
                                nc.gpsimd.dma_start(out=at[:, 0:128], in_=adjs[r0:r0 + 128, 0:128])
                            else:
                                nc.gpsimd.dma_start(out=at[:], in_=adjs[r0:r0 + 128, :])
                        strips.append(at)
                    Us = [
                        upsp.tile([128, XW], f32, tag="U", name=f"U_{p}_{i}")
                        for i in range(4)
                    ]
                    for jt4 in range(NJT // 4):
                        m_t = mwp.tile([128, 2048], f32, tag="m")
                        for h in range(4):
                            jt = jt4 * 4 + h
                            aT = atp.tile([128, PANEL], bf16, tag="aT")
                            if "nopet" not in ablate:
                              for s in range(4):
                                nc.tensor.transpose(
                                    aT[:, s * 128:(s + 1) * 128],
                                    strips[s][:, jt * 128:(jt + 1) * 128],
                                    ident_b[:],
                                )
                            if "nodve" in ablate:
                                continue
                            nc.vector._custom_dve(
                                GAT_OP,
                                out=m_t[:, h * PANEL:(h + 1) * PANEL],
                                in0=src_bc[:, p * PANEL:(p + 1) * PANEL],
                                in1=aT[:],
                                s0=dst_all[:, jt:jt + 1],
                                s1=NEG_SLOPE,
                            )
                        w_t = wxp.tile([128, 2048], bf16, tag="w")
                        if "noact" not in ablate:
                            nc.scalar.activation(w_t[:], m_t[:], AF.Exp)
                        if "noagg" in ablate:
                            continue
                        for h in range(4):
                            jt = jt4 * 4 + h
                            for ic in range(4):
                                nc.tensor.matmul(
                                    Us[ic][:],
                                    w_t[:, h * PANEL + ic * 128: h * PANEL + (ic + 1) * 128],
                                    xt_all[:, jt * XW:(jt + 1) * XW],
                                    start=(jt == 0),
                                    stop=(jt == NJT - 1),
                                )
                    if "noagg" in ablate:
                        continue
                    o_t = finp.tile([128, PANEL], f32, tag="o")
                    for ic in range(4):
                        rec = finp.tile([128, 1], f32, tag="rec")
                        nc.vector.reciprocal(rec[:], Us[ic][:, HC:HC + 1])
                        nc.vector.tensor_scalar_mul(
                            o_t[:, ic * HC:(ic + 1) * HC], Us[ic][:, 0:HC], rec[:]
                        )
                    r0 = p * PANEL
                    nc.sync.dma_start(
                        out=outd[r0:r0 + PANEL, :].rearrange("(k p) d -> p k d", p=128),
                        in_=o_t[:].rearrange("p (k d) -> p k d", k=4),
                    )

    nc.compile()
    _CACHE[key] = nc
    return nc


def _in_maps(regional_means, adj, W, b, a):
    regional_means = np.ascontiguousarray(regional_means, dtype=np.float32)
    adj = np.ascontiguousarray(adj, dtype=np.float32)
    W = np.ascontiguousarray(W, dtype=np.float32)
    b = np.asarray(b, dtype=np.float32)
    a = np.asarray(a, dtype=np.float32)
    a_src, a_dst = a[:HC], a[HC:]
    brow = np.concatenate([b, [1.0, 0.0]]).reshape(1, XW + 1).astype(np.float32)
    WT = np.ascontiguousarray(W.T)                       # (Cin, HC)
    wv = np.ascontiguousarray(W.T @ np.stack([a_src, a_dst], 1))  # (Cin, 2)
    cc = np.array([[b @ (a_src + a_dst)]], dtype=np.float32)
    maps = []
    for c in range(NCORES):
        bb, hf = divmod(c, 2)
        i0 = hf * ROWS
        rmT = np.ascontiguousarray(regional_means[bb].T)  # (Cin, N)
        maps.append(
            {
                "rmT": rmT,
                "rmoT": np.ascontiguousarray(rmT[:, i0:i0 + ROWS]),
                "adjs": np.ascontiguousarray(adj[bb, i0:i0 + ROWS]),
                "WTd": WT,
                "wvd": wv,
                "brow": brow,
                "cscal": cc,
            }
        )
    return maps


def kernel(regional_means, adj, W, b, a):
    from concourse.bass_utils import run_bass_kernel_spmd

    nc = _build()
    maps = _in_maps(regional_means, adj, W, b, a)
    res = run_bass_kernel_spmd(nc, maps, core_ids=list(range(NCORES)))
    out = np.empty((B, N, HC), np.float32)
    for c in range(NCORES):
        bb, hf = divmod(c, 2)
        out[bb, hf * ROWS:(hf + 1) * ROWS] = res.results[c]["out"]
    return out
